# revision 9
# baseline (speedup 1.0000x reference)
"""Causal multi-head attention block (b=8, s=1024, d_model=768, 12 heads x 64)
on 8 TRN2 NeuronCores — batch-parallel: core i computes batch element i.

Self-contained: includes the NTFF-profile-hook shim and the BIR wait-split
workaround for this walrus build (max 1 semaphore wait per instruction).

Per-core plan (bf16 matmuls, fp32 PSUM accumulation):
  A. x arrives bf16 (host-transposed); x / W_Q land in two big DMAs each on
     the sync (HWDGE) queue, W_V/W_K/W_O on the scalar queue, biases on
     gpsimd — few large transfers, issue-order = first-use order.
  B. QT/KT [hd-blk][128,1024] = W.T @ xT (head-pair packed); V in natural
     [s,hd] layout padded to 65 cols/head with a ones column (rowsum trick)
  C. per q-half(512) / head-pair: scoresT[k,q] = KT.T @ QT on PE (left-
     trimmed to the causal window), exp on ACT (1/8 scale folded in),
     causal mask via gpsimd affine_select on [128,128] diagonal windows,
     PV accumulates [65,512]x2 (row 64 = softmax denominator).
     Denominators land at partitions {0,32} of a [33,3072] tile; one
     batched Ln+Exp pair per q-half computes reciprocals; K=1 ones-row
     matmuls broadcast them and DVE multiplies normalize the stacks.
  D. out-proj + b_O in bf16; y DMA'd bf16 (host upcasts to f32).
  Attention is the emission backbone from the start; every other matmul
  (projections, out-proj, normalization broadcast) is doled out by a
  feeder between attention k-tiles so the in-order PE queue always holds
  runnable work — keeping the PE HAM clock-gate warm through the
  ACT-paced stretches.
"""

import os
import sys
import types

import numpy as np

# ---------------------------------------------------------------------------
# environment shims


def _install_ntff_hook():
    try:
        import antenv
        from trn_agent_boot.trn_boot import _ntff_profile_via_ctypes
    except Exception:
        return
    if "antenv.axon_hooks" in sys.modules:
        return
    hook = _ntff_profile_via_ctypes("/opt/axon/libaxon_pjrt.so")
    m = types.ModuleType("antenv.axon_hooks")
    m.set_axon_ntff_profile_hook = lambda h: None
    m.get_axon_ntff_profile_hook = lambda: hook
    sys.modules["antenv.axon_hooks"] = m
    antenv.axon_hooks = m


def _install_waitsplit(max_waits=1):
    """walrus on this build rejects >1 sem wait per instruction; split extras
    onto preceding NoOps (same engine, program order preserved)."""
    import json

    import concourse.bass as bass

    if getattr(bass.Bass, "_waitsplit_installed", False):
        return
    counter = [0]

    def _split(inst):
        si = inst.get("sync_info")
        if not si:
            return [inst]
        waits = si.get("on_wait") or []
        if len(waits) <= max_waits:
            return [inst]
        out = []
        head, rest = waits[:-max_waits], waits[-max_waits:]
        for i in range(0, len(head), max_waits):
            counter[0] += 1
            out.append(
                {
                    "debug": inst.get("debug", 0),
                    "engine": inst["engine"],
                    "ins": [],
                    "name": f"I-waitsplit-{counter[0]}",
                    "opcode": "NoOp",
                    "outs": [],
                    "text_hint": "waitsplit",
                    "sync_info": {
                        "on_update": [],
                        "on_wait": head[i : i + max_waits],
                    },
                }
            )
        si["on_wait"] = rest
        out.append(inst)
        return out

    orig = bass.Bass.to_json_bytes

    def to_json_bytes(self):
        d = json.loads(orig(self))
        changed = False
        for f in d.get("functions", []):
            for bb in f.get("blocks", []):
                new = []
                for inst in bb.get("instructions", []):
                    parts = _split(inst)
                    changed = changed or len(parts) > 1
                    new.extend(parts)
                bb["instructions"] = new
        return json.dumps(d).encode() if changed else orig(self)

    bass.Bass.to_json_bytes = to_json_bytes
    bass.Bass._waitsplit_installed = True


_install_ntff_hook()
_install_waitsplit()

import ml_dtypes  # noqa: E402
import concourse.bass as bass  # noqa: E402
import concourse.mybir as mybir  # noqa: E402
import concourse.tile as tile  # noqa: E402
from concourse.bass_utils import run_bass_kernel_spmd  # noqa: E402

# ---------------------------------------------------------------------------
# problem constants (hardcoded per harness contract)

B, S, D, H, DH = 8, 1024, 768, 12, 64
P = 128
MT = D // P            # 6 tiles over d_model / hd
QC = 256               # q-chunk width
QH = 512               # q-half (pair of chunks)
NKT = S // P           # 8 k-tiles over seq
SCALE = float(1.0 / np.sqrt(DH))
N_CORES = 8

F32 = mybir.dt.float32
F32R = mybir.dt.float32r
BF16 = mybir.dt.bfloat16
MMDT = BF16


def build_nc() -> bass.Bass:
    nc = bass.Bass()
    xT = nc.declare_dram_parameter("xT", [D, S], MMDT, isOutput=False)
    wq = nc.declare_dram_parameter("wq", [D, D], MMDT, isOutput=False)
    wk = nc.declare_dram_parameter("wk", [D, D], MMDT, isOutput=False)
    wv = nc.declare_dram_parameter("wv", [D, D], MMDT, isOutput=False)
    wo = nc.declare_dram_parameter("wo", [D, D], MMDT, isOutput=False)
    bq = nc.declare_dram_parameter("bq", [D], F32, isOutput=False)
    bk = nc.declare_dram_parameter("bk", [D], F32, isOutput=False)
    bv = nc.declare_dram_parameter("bv", [D], F32, isOutput=False)
    bo = nc.declare_dram_parameter("bo", [D], F32, isOutput=False)
    y = nc.declare_dram_parameter("y", [S, D], MMDT, isOutput=True)

    Exp = mybir.ActivationFunctionType.Exp
    Ln = mybir.ActivationFunctionType.Ln
    mult = mybir.AluOpType.mult
    add = mybir.AluOpType.add
    is_ge = mybir.AluOpType.is_ge

    from contextlib import ExitStack

    with ExitStack() as _ctx:
        tc = _ctx.enter_context(tile.TileContext(nc))
        constp = _ctx.enter_context(tc.tile_pool(name="const", bufs=1))
        xtp = _ctx.enter_context(tc.tile_pool(name="xT", bufs=1))
        qtp = _ctx.enter_context(tc.tile_pool(name="qt", bufs=1))
        ktp = _ctx.enter_context(tc.tile_pool(name="kt", bufs=1))
        vpp = _ctx.enter_context(tc.tile_pool(name="vp", bufs=1))
        wtsp = _ctx.enter_context(tc.tile_pool(name="wts", bufs=1))
        expp = _ctx.enter_context(tc.tile_pool(name="expst", bufs=10))
        wsp = _ctx.enter_context(tc.tile_pool(name="wstack", bufs=12))
        outp = _ctx.enter_context(tc.tile_pool(name="outsb", bufs=2))
        psflow = _ctx.enter_context(
            tc.tile_pool(name="ps_flow", bufs=1, space="PSUM")
        )
        psacc = _ctx.enter_context(
            tc.tile_pool(name="ps_acc", bufs=3, space="PSUM")
        )
        scpp = _ctx.enter_context(
            tc.tile_pool(name="ps_scp", bufs=2, space="PSUM")
        )

        # ---- input DMAs: few and large, first-use order ---------------------
        xt_all = xtp.tile([P, MT * S], MMDT, tag="xta", name="xta")
        xt3 = xt_all.rearrange("p (t s) -> p t s", s=S)
        xTv = xT.rearrange("(t p) s -> p t s", p=P)
        nc.sync.dma_start(xt3[:, 0:2, :], xTv[:, 0:2, :])
        nc.sync.dma_start(xt3[:, 2:4, :], xTv[:, 2:4, :])
        nc.sync.dma_start(xt3[:, 4:6, :], xTv[:, 4:6, :])

        def wtile(name):
            t = wtsp.tile([P, MT * D], MMDT, tag=name, name=name)
            return t, t.rearrange("p (t c) -> p t c", c=D)

        wq_all, wq3 = wtile("wqt")
        wk_all, wk3 = wtile("wkt")
        wv_all, wv3 = wtile("wvt")
        wo_all, wo3 = wtile("wot")

        def wload(eng, t3, dram):
            dv = dram.rearrange("(t p) c -> p t c", p=P)
            eng.dma_start(t3[:, 0:3, :], dv[:, 0:3, :])
            eng.dma_start(t3[:, 3:6, :], dv[:, 3:6, :])

        wload(nc.sync, wq3, wq)
        wload(nc.scalar, wv3, wv)
        wload(nc.scalar, wk3, wk)
        wload(nc.scalar, wo3, wo)

        # gpsimd queue: bias constants (one strided DMA per bias vector)
        bq_t = constp.tile([P, MT], F32, tag="bq")  # col hdb = bias block
        bk_t = constp.tile([P, MT], F32, tag="bk")
        nc.gpsimd.dma_start(bq_t[:], bq.rearrange("(t p) -> p t", p=P))
        nc.gpsimd.dma_start(bk_t[:], bk.rearrange("(t p) -> p t", p=P))
        bv_stage = constp.tile([1, D], F32, tag="bstage", bufs=2, name="bv_stage")
        nc.gpsimd.dma_start(bv_stage[:], bv.rearrange("(o d) -> o d", o=1))
        bo_stage = constp.tile([1, D], F32, tag="bstage", bufs=2, name="bo_stage")
        nc.gpsimd.dma_start(bo_stage[:], bo.rearrange("(o d) -> o d", o=1))

        # warm the ACT table set (exp+ln) while DMAs stream
        actw = constp.tile([1, 8], F32, tag="actw")
        nc.vector.memset(actw[:], 1.0)
        nc.scalar.activation(actw[0:1, 0:4], actw[0:1, 4:8], Ln)
        nc.scalar.activation(actw[0:1, 0:4], actw[0:1, 4:8], Exp)

        # ---- small on-chip constants ---------------------------------------
        ones_stage = constp.tile([1, P], F32, tag="onesstage")
        nc.vector.memset(ones_stage[:], 1.0)
        ones_row = constp.tile([1, P], F32R, tag="onesrow")
        nc.vector.tensor_copy(ones_row[:], ones_stage[:])

        bv_row = constp.tile([1, D], F32R, tag="bvrow")
        nc.vector.tensor_copy(bv_row[:], bv_stage[:])
        bo_row = constp.tile([1, D], F32R, tag="borow")
        nc.vector.tensor_copy(bo_row[:], bo_stage[:])

        ones_col = constp.tile([P, H], F32, tag="onescol")
        nc.vector.memset(ones_col[:], 1.0)

        # all-ones F32R rows (only partitions 0/32 are read) for denom bcast
        ones33_stage = constp.tile([33, 64], F32, tag="o33s")
        nc.vector.memset(ones33_stage[:], 1.0)
        ones33 = constp.tile([33, 64], F32R, tag="o33")
        nc.vector.tensor_copy(ones33[:], ones33_stage[:])

        # denominator staging: engine APs need 32-aligned partition bases,
        # so head (2hp+sub)'s rowsums live at [32*sub, hp*QH:(hp+1)*QH].
        # Rows 1..31 are never read; memset keeps Ln's input defined.
        DEN_W = MT * QH
        denoms = [
            constp.tile([33, DEN_W], F32, tag=f"den{pp}", name=f"den{pp}")
            for pp in range(2)
        ]
        for pp in range(2):
            nc.vector.memset(denoms[pp][:], 1.0)
        lnr_t = constp.tile([33, DEN_W], F32, tag="lnrt", name="lnrt")
        frec_t = constp.tile([33, DEN_W], F32R, tag="frect", name="frect")

        # ---- broadcast bias rows to all partitions via K=1 matmul ----------
        bv_b = constp.tile([P, D], F32, tag="bvb")
        bo_b = constp.tile([P, D], F32, tag="bob")

        def bias_bcast(row, bcast):
            for c0, c1 in ((0, 512), (512, 768)):
                bps = psflow.tile([P, 512], F32, tag="ps", name="bps")
                nc.tensor.matmul(
                    bps[:, : c1 - c0],
                    ones_row[:],
                    row[:, c0:c1],
                    start=True,
                    stop=True,
                )
                nc.vector.tensor_copy(bcast[:, c0:c1], bps[:, : c1 - c0])

        # ---- projections ----------------------------------------------------
        qts = [qtp.tile([P, S], MMDT, tag=f"qt{i}", name=f"qt{i}") for i in range(MT)]
        kts = [ktp.tile([P, S], MMDT, tag=f"kt{i}", name=f"kt{i}") for i in range(MT)]
        vps = [
            vpp.tile([P, H * 65], MMDT, tag=f"vp{st}", name=f"vp{st}")
            for st in range(NKT)
        ]

        def proj_qk_gen(w3, b_t, dst, sc, hdb):
            s0 = sc * 512
            ps0 = psflow.tile([P, 512], F32, tag="ps", name="pj0")
            for mt in range(MT):
                nc.tensor.matmul(
                    ps0[:], w3[:, mt, hdb * P : (hdb + 1) * P],
                    xt3[:, mt, s0 : s0 + 512],
                    start=(mt == 0), stop=(mt == MT - 1),
                )
                if mt in (1, 3):
                    yield
            bsl = b_t[:, hdb : hdb + 1]
            bb = bass.AP(bsl.tensor, bsl.offset, [bsl.ap[0], [0, 512]])
            nc.vector.tensor_tensor(
                dst[hdb][:, s0 : s0 + 512], ps0[:], bb, op=add
            )

        def proj_qk_piece(w3, b_t, dst, sc, hdb):
            for _ in proj_qk_gen(w3, b_t, dst, sc, hdb):
                pass

        def proj_v_gen(st):
            vv = vps[st].rearrange("p (h c) -> p h c", c=65)
            nc.vector.tensor_copy(
                vv[:, :, 64:65],
                ones_col.rearrange("p (h c) -> p h c", c=1),
            )
            bsrc = bv_b.rearrange("p (h c) -> p h c", c=DH)
            ps0 = psflow.tile([P, 512], F32, tag="ps", name="pv0")
            for mt in range(MT):
                nc.tensor.matmul(
                    ps0[:], xt3[:, mt, st * P : (st + 1) * P],
                    wv3[:, mt, 0:512],
                    start=(mt == 0), stop=(mt == MT - 1),
                )
                if mt in (1, 3):
                    yield
            nc.vector.tensor_tensor(
                vv[:, 0:8, 0:DH],
                ps0.rearrange("p (h c) -> p h c", c=DH),
                bsrc[:, 0:8, :],
                op=add,
            )
            yield
            ps1 = psflow.tile([P, 512], F32, tag="ps", name="pv1")
            for mt in range(MT):
                nc.tensor.matmul(
                    ps1[:, 0:256], xt3[:, mt, st * P : (st + 1) * P],
                    wv3[:, mt, 512:768],
                    start=(mt == 0), stop=(mt == MT - 1),
                )
                if mt == 3:
                    yield
            nc.vector.tensor_tensor(
                vv[:, 8:12, 0:DH],
                ps1[:, 0:256].rearrange("p (h c) -> p h c", c=DH),
                bsrc[:, 8:12, :],
                op=add,
            )

        class Feeder:
            """Doles out deferred emission work in small steps so the PE
            stream interleaves finely with attention matmuls."""

            def __init__(self):
                from collections import deque
                self.q = deque()

            def add(self, gen):
                self.q.append(gen)
                return gen

            def step(self):
                while self.q:
                    try:
                        next(self.q[0])
                        return True
                    except StopIteration:
                        self.q.popleft()
                return False

            def drain_until(self, gen):
                """Emit queued work until `gen` has fully emitted."""
                while any(g is gen for g in self.q):
                    try:
                        next(self.q[0])
                    except StopIteration:
                        self.q.popleft()

            def drain(self):
                while self.step():
                    pass

        feeder = Feeder()

        def attn_core_gen(pp, hp, wstack):
            q0 = pp * QH
            nkt1 = 4 * pp + 4
            pvs = [
                psacc.tile([65, QH], F32, tag="pv", name=f"pv{sub}")
                for sub in range(2)
            ]
            for kt in range(nkt1):
                # left edge of the causal window at 128-col granularity
                c0 = max(0, kt - 4 * pp) * P
                w = QH - c0
                scp = scpp.tile([P, 2 * QH], F32, tag="scp", name="scp")
                # the pair's two matmuls sit on disjoint PE row groups and
                # disjoint PSUM banks of one 2-bank tile
                for sub in range(2):
                    r0 = sub * 64
                    nc.tensor.matmul(
                        scp[:, sub * QH + c0 : (sub + 1) * QH],
                        kts[hp][r0 : r0 + 64, kt * P : (kt + 1) * P],
                        qts[hp][r0 : r0 + 64, q0 + c0 : q0 + QH],
                        start=True,
                        stop=True,
                        tile_position=(r0, 0),
                    )
                est = expp.tile([P, 2 * QH], MMDT, tag="est", name="est")
                if c0 == 0:
                    nc.scalar.activation(
                        est[:], scp[:], Exp, scale=SCALE
                    )
                else:
                    sin = bass.AP(
                        scp.tensor, scp.offset + c0,
                        [scp.ap[0], [QH, 2], [1, w]],
                    )
                    sout = bass.AP(
                        est.tensor, est.offset + c0,
                        [est.ap[0], [QH, 2], [1, w]],
                    )
                    nc.scalar.activation(sout, sin, Exp, scale=SCALE)
                dk = kt - 4 * pp
                if dk >= 0:
                    ms = dk * P
                    for sub in range(2):
                        b0 = sub * QH
                        nc.gpsimd.affine_select(
                            est[:, b0 + ms : b0 + ms + P],
                            est[:, b0 + ms : b0 + ms + P],
                            pattern=[[1, P]],
                            compare_op=is_ge, fill=0.0,
                            base=0,
                            channel_multiplier=-1,
                        )
                for sub in range(2):
                    h = 2 * hp + sub
                    nc.tensor.matmul(
                        pvs[sub][:, c0:QH],
                        vps[kt][:, h * 65 : (h + 1) * 65],
                        est[:, sub * QH + c0 : (sub + 1) * QH],
                        start=(kt == 0),
                        stop=(kt == nkt1 - 1),
                        skip_group_check=True,
                    )
                yield
            # stash: unnormalized rows into wstack, denominator rows into
            # the batched denom tile (partition 32*sub, col block hp)
            for sub in range(2):
                r0 = sub * 64
                nc.vector.tensor_copy(
                    wstack[hp][r0 : r0 + 64, :], pvs[sub][0:64, :]
                )
                nc.vector.tensor_copy(
                    denoms[pp][32 * sub : 32 * sub + 1,
                               hp * QH : (hp + 1) * QH],
                    pvs[sub][64:65, :],
                )

        def run_attn(pp, hp, wstack, steps=1):
            for _ in attn_core_gen(pp, hp, wstack):
                for _ in range(steps):
                    feeder.step()

        def norm_apply(pp, hp, wstack):
            for sub in range(2):
                r0 = sub * 64
                rb = psflow.tile([P, 512], F32, tag="ps", name="rb")
                nc.tensor.matmul(
                    rb[0:64, :],
                    ones33[32 * sub : 32 * sub + 1, :],
                    frec_t[32 * sub : 32 * sub + 1,
                           hp * QH : (hp + 1) * QH],
                    start=True, stop=True,
                )
                nc.vector.tensor_tensor(
                    wstack[hp][r0 : r0 + 64, :],
                    wstack[hp][r0 : r0 + 64, :],
                    rb[0:64, :], op=mult,
                )

        def norm_gen(pp, h0, h1, wstack):
            # reciprocal via exp(-ln r) — same ACT table set as softmax exp
            c0, c1 = h0 * QH, h1 * QH
            nc.scalar.activation(
                lnr_t[:, c0:c1], denoms[pp][:, c0:c1], Ln
            )
            nc.scalar.activation(
                frec_t[:, c0:c1], lnr_t[:, c0:c1], Exp, scale=-1.0
            )
            yield
            for hp in range(h0, h1):
                norm_apply(pp, hp, wstack)
                yield

        def outproj_gen(pp, wstack, sub):
            q0 = pp * QH
            osb = outp.tile([P, D], MMDT, tag="osb")
            opsa = psflow.tile([P, 512], F32, tag="ps", name="opa_t")
            for hdt in range(MT):
                nc.tensor.matmul(
                    opsa[:], wstack[hdt][:, sub * P : (sub + 1) * P],
                    wo3[:, hdt, 0:512],
                    start=(hdt == 0), stop=(hdt == MT - 1),
                )
                if hdt in (1, 3):
                    yield
            nc.vector.tensor_tensor(
                osb[:, 0:512], opsa[:], bo_b[:, 0:512], op=add
            )
            yield
            opsb = psflow.tile([P, 512], F32, tag="ps", name="opb_t")
            for hdt in range(MT):
                nc.tensor.matmul(
                    opsb[:, 0:256], wstack[hdt][:, sub * P : (sub + 1) * P],
                    wo3[:, hdt, 512:768],
                    start=(hdt == 0), stop=(hdt == MT - 1),
                )
                if hdt == 3:
                    yield
            nc.vector.tensor_tensor(
                osb[:, 512:768], opsb[:, 0:256], bo_b[:, 512:768], op=add
            )
            eng = nc.sync if (pp == 0 or sub % 2 == 0) else nc.gpsimd
            eng.dma_start(y[q0 + sub * P : q0 + (sub + 1) * P, :], osb[:])

        # ---- emission: attention as backbone, all else fed between k-tiles
        wstack0 = [
            wsp.tile([P, QH], MMDT, tag="ws", name=f"ws0_{i}")
            for i in range(MT)
        ]
        wstack1 = [
            wsp.tile([P, QH], MMDT, tag="ws", name=f"ws1_{i}")
            for i in range(MT)
        ]

        # prelude: exactly what attn(0,0) needs
        proj_qk_piece(wq3, bq_t, qts, 0, 0)
        bias_bcast(bv_row, bv_b)
        bias_bcast(bo_row, bo_b)
        proj_qk_piece(wk3, bk_t, kts, 0, 0)
        for st in range(4):
            for _ in proj_v_gen(st):
                pass

        k0_gens = {}
        k1_gens = {}
        for hp in range(MT):
            if hp < MT - 1:
                feeder.add(proj_qk_gen(wq3, bq_t, qts, 0, hp + 1))
                k0_gens[hp + 1] = feeder.add(
                    proj_qk_gen(wk3, bk_t, kts, 0, hp + 1)
                )
            if hp < 4:
                feeder.add(proj_v_gen(4 + hp))
            if hp in k0_gens:
                feeder.drain_until(k0_gens[hp])
            run_attn(0, hp, wstack0, steps=2)

        # stage half-1 projections for the first two head-pairs, then let
        # the pp1 attention stream pace the rest
        feeder.add(proj_qk_gen(wq3, bq_t, qts, 1, 0))
        k1_gens[0] = feeder.add(proj_qk_gen(wk3, bk_t, kts, 1, 0))
        feeder.add(proj_qk_gen(wq3, bq_t, qts, 1, 1))
        k1_gens[1] = feeder.add(proj_qk_gen(wk3, bk_t, kts, 1, 1))
        feeder.drain_until(k1_gens[0])

        for hp in range(MT):
            if hp < 4:
                feeder.add(proj_qk_gen(wq3, bq_t, qts, 1, hp + 2))
                k1_gens[hp + 2] = feeder.add(
                    proj_qk_gen(wk3, bk_t, kts, 1, hp + 2)
                )
            if hp == 0:
                feeder.add(norm_gen(0, 0, MT, wstack0))
            elif hp <= 4:
                feeder.add(outproj_gen(0, wstack0, hp - 1))
            else:
                feeder.add(norm_gen(1, 0, 5, wstack1))
            feeder.drain_until(k1_gens[hp])
            run_attn(1, hp, wstack1, steps=1)

        feeder.drain()
        for _ in norm_gen(1, 5, MT, wstack1):
            pass
        for sub in range(4):
            for _ in outproj_gen(1, wstack1, sub):
                pass
    return nc


_NC_CACHE = None
LAST_EXEC_NS = None


def _get_nc():
    global _NC_CACHE
    if _NC_CACHE is None:
        _NC_CACHE = build_nc()
    return _NC_CACHE


def kernel(
    normalized_resid_pre, W_Q, W_K, W_V, W_O, b_Q, b_K, b_V, b_O
) -> np.ndarray:
    global LAST_EXEC_NS
    bf = ml_dtypes.bfloat16
    x = np.asarray(normalized_resid_pre, np.float32)
    xT = np.ascontiguousarray(x.transpose(0, 2, 1)).astype(bf)  # [b, D, S]
    wq = np.asarray(W_Q, np.float32).transpose(1, 0, 2).reshape(D, D).astype(bf)
    wk = np.asarray(W_K, np.float32).transpose(1, 0, 2).reshape(D, D).astype(bf)
    wv = np.asarray(W_V, np.float32).transpose(1, 0, 2).reshape(D, D).astype(bf)
    wo = np.asarray(W_O, np.float32).reshape(D, D).astype(bf)
    bq = np.asarray(b_Q, np.float32).reshape(D).copy()
    bk = np.asarray(b_K, np.float32).reshape(D).copy()
    bv = np.asarray(b_V, np.float32).reshape(D).copy()
    bo = np.asarray(b_O, np.float32).reshape(D).copy()

    nc = _get_nc()
    in_maps = [
        {
            "xT": xT[i],
            "wq": wq, "wk": wk, "wv": wv, "wo": wo,
            "bq": bq, "bk": bk, "bv": bv, "bo": bo,
        }
        for i in range(N_CORES)
    ]
    trace = os.environ.get("KERNEL_TRACE", "0") == "1"
    res = run_bass_kernel_spmd(
        nc, in_maps, list(range(N_CORES)), trace=trace
    )
    LAST_EXEC_NS = res.exec_time_ns
    out = np.stack(
        [res.results[i]["y"].astype(np.float32) for i in range(N_CORES)], axis=0
    )
    return out


# revision 10
# speedup vs baseline: 1.1323x; 1.1323x over previous
"""Causal multi-head attention block (b=8, s=1024, d_model=768, 12 heads x 64)
on 8 TRN2 NeuronCores — batch-parallel: core i computes batch element i.

Self-contained: includes the NTFF-profile-hook shim and the BIR wait-split
workaround for this walrus build (max 1 semaphore wait per instruction).

Per-core plan (bf16 matmuls, fp32 PSUM accumulation):
  A. x arrives bf16 (host-transposed); x / W_Q land in two big DMAs each on
     the sync (HWDGE) queue, W_V/W_K/W_O on the scalar queue, biases on
     gpsimd — few large transfers, issue-order = first-use order.
  B. QT/KT [hd-blk][128,1024] = W.T @ xT (head-pair packed); V in natural
     [s,hd] layout padded to 65 cols/head with a ones column (rowsum trick)
  C. per q-half(512) / head-pair: scoresT[k,q] = KT.T @ QT on PE (left-
     trimmed to the causal window), exp on ACT (1/8 scale folded in),
     causal mask via gpsimd affine_select on [128,128] diagonal windows,
     PV accumulates [65,512]x2 (row 64 = softmax denominator).
     Denominators land at partitions {0,32} of a [33,3072] tile; one
     batched Ln+Exp pair per q-half computes reciprocals; K=1 ones-row
     matmuls broadcast them and DVE multiplies normalize the stacks.
  D. out-proj + b_O in bf16; y DMA'd bf16 (host upcasts to f32).
  Attention is the emission backbone from the start; every other matmul
  (projections, out-proj, normalization broadcast) is doled out by a
  feeder between attention k-tiles so the in-order PE queue always holds
  runnable work — keeping the PE HAM clock-gate warm through the
  ACT-paced stretches.
"""

import os
import sys
import types

import numpy as np

# ---------------------------------------------------------------------------
# environment shims


def _install_ntff_hook():
    try:
        import antenv
        from trn_agent_boot.trn_boot import _ntff_profile_via_ctypes
    except Exception:
        return
    if "antenv.axon_hooks" in sys.modules:
        return
    hook = _ntff_profile_via_ctypes("/opt/axon/libaxon_pjrt.so")
    m = types.ModuleType("antenv.axon_hooks")
    m.set_axon_ntff_profile_hook = lambda h: None
    m.get_axon_ntff_profile_hook = lambda: hook
    sys.modules["antenv.axon_hooks"] = m
    antenv.axon_hooks = m


def _install_waitsplit(max_waits=1):
    """walrus on this build rejects >1 sem wait per instruction; split extras
    onto preceding NoOps (same engine, program order preserved)."""
    import json

    import concourse.bass as bass

    if getattr(bass.Bass, "_waitsplit_installed", False):
        return
    counter = [0]

    def _split(inst):
        si = inst.get("sync_info")
        if not si:
            return [inst]
        waits = si.get("on_wait") or []
        if len(waits) <= max_waits:
            return [inst]
        out = []
        head, rest = waits[:-max_waits], waits[-max_waits:]
        for i in range(0, len(head), max_waits):
            counter[0] += 1
            out.append(
                {
                    "debug": inst.get("debug", 0),
                    "engine": inst["engine"],
                    "ins": [],
                    "name": f"I-waitsplit-{counter[0]}",
                    "opcode": "NoOp",
                    "outs": [],
                    "text_hint": "waitsplit",
                    "sync_info": {
                        "on_update": [],
                        "on_wait": head[i : i + max_waits],
                    },
                }
            )
        si["on_wait"] = rest
        out.append(inst)
        return out

    orig = bass.Bass.to_json_bytes

    def to_json_bytes(self):
        d = json.loads(orig(self))
        changed = False
        for f in d.get("functions", []):
            for bb in f.get("blocks", []):
                new = []
                for inst in bb.get("instructions", []):
                    parts = _split(inst)
                    changed = changed or len(parts) > 1
                    new.extend(parts)
                bb["instructions"] = new
        return json.dumps(d).encode() if changed else orig(self)

    bass.Bass.to_json_bytes = to_json_bytes
    bass.Bass._waitsplit_installed = True


_install_ntff_hook()
_install_waitsplit()

import ml_dtypes  # noqa: E402
import concourse.bass as bass  # noqa: E402
import concourse.mybir as mybir  # noqa: E402
import concourse.tile as tile  # noqa: E402
from concourse.bass_utils import run_bass_kernel_spmd  # noqa: E402

# ---------------------------------------------------------------------------
# problem constants (hardcoded per harness contract)

B, S, D, H, DH = 8, 1024, 768, 12, 64
P = 128
MT = D // P            # 6 tiles over d_model / hd
QC = 256               # q-chunk width
QH = 512               # q-half (pair of chunks)
NKT = S // P           # 8 k-tiles over seq
SCALE = float(1.0 / np.sqrt(DH))
N_CORES = 8

F32 = mybir.dt.float32
F32R = mybir.dt.float32r
BF16 = mybir.dt.bfloat16
MMDT = BF16


def build_nc() -> bass.Bass:
    nc = bass.Bass()
    xT = nc.declare_dram_parameter("xT", [D, S], MMDT, isOutput=False)
    wq = nc.declare_dram_parameter("wq", [D, D], MMDT, isOutput=False)
    wk = nc.declare_dram_parameter("wk", [D, D], MMDT, isOutput=False)
    wv = nc.declare_dram_parameter("wv", [D, D], MMDT, isOutput=False)
    wo = nc.declare_dram_parameter("wo", [D, D], MMDT, isOutput=False)
    bq = nc.declare_dram_parameter("bq", [D], F32, isOutput=False)
    bk = nc.declare_dram_parameter("bk", [D], F32, isOutput=False)
    bv = nc.declare_dram_parameter("bv", [D], F32, isOutput=False)
    bo = nc.declare_dram_parameter("bo", [D], F32, isOutput=False)
    y = nc.declare_dram_parameter("y", [S, D], MMDT, isOutput=True)

    Exp = mybir.ActivationFunctionType.Exp
    Ln = mybir.ActivationFunctionType.Ln
    mult = mybir.AluOpType.mult
    add = mybir.AluOpType.add
    is_ge = mybir.AluOpType.is_ge

    from contextlib import ExitStack

    with ExitStack() as _ctx:
        tc = _ctx.enter_context(tile.TileContext(nc))
        constp = _ctx.enter_context(tc.tile_pool(name="const", bufs=1))
        xtp = _ctx.enter_context(tc.tile_pool(name="xT", bufs=1))
        qtp = _ctx.enter_context(tc.tile_pool(name="qt", bufs=1))
        ktp = _ctx.enter_context(tc.tile_pool(name="kt", bufs=1))
        vpp = _ctx.enter_context(tc.tile_pool(name="vp", bufs=1))
        wtsp = _ctx.enter_context(tc.tile_pool(name="wts", bufs=1))
        expp = _ctx.enter_context(tc.tile_pool(name="expst", bufs=10))
        wsp = _ctx.enter_context(tc.tile_pool(name="wstack", bufs=12))
        outp = _ctx.enter_context(tc.tile_pool(name="outsb", bufs=2))
        psflow = _ctx.enter_context(
            tc.tile_pool(name="ps_flow", bufs=2, space="PSUM")
        )
        psacc = _ctx.enter_context(
            tc.tile_pool(name="ps_acc", bufs=2, space="PSUM")
        )
        scpp = _ctx.enter_context(
            tc.tile_pool(name="ps_scp", bufs=2, space="PSUM")
        )

        # ---- input DMAs: few and large, first-use order ---------------------
        xt_all = xtp.tile([P, MT * S], MMDT, tag="xta", name="xta")
        xt3 = xt_all.rearrange("p (t s) -> p t s", s=S)
        xTv = xT.rearrange("(t p) s -> p t s", p=P)
        nc.sync.dma_start(xt3[:, 0:2, :], xTv[:, 0:2, :])
        nc.sync.dma_start(xt3[:, 2:4, :], xTv[:, 2:4, :])
        nc.sync.dma_start(xt3[:, 4:6, :], xTv[:, 4:6, :])

        def wtile(name):
            t = wtsp.tile([P, MT * D], MMDT, tag=name, name=name)
            return t, t.rearrange("p (t c) -> p t c", c=D)

        wq_all, wq3 = wtile("wqt")
        wk_all, wk3 = wtile("wkt")
        wv_all, wv3 = wtile("wvt")
        wo_all, wo3 = wtile("wot")

        def wload(eng, t3, dram):
            dv = dram.rearrange("(t p) c -> p t c", p=P)
            eng.dma_start(t3[:, 0:3, :], dv[:, 0:3, :])
            eng.dma_start(t3[:, 3:6, :], dv[:, 3:6, :])

        wload(nc.sync, wq3, wq)
        wload(nc.scalar, wv3, wv)
        wload(nc.scalar, wk3, wk)
        wload(nc.scalar, wo3, wo)

        # gpsimd queue: bias constants (one strided DMA per bias vector)
        bq_t = constp.tile([P, MT], F32, tag="bq")  # col hdb = bias block
        bk_t = constp.tile([P, MT], F32, tag="bk")
        nc.gpsimd.dma_start(bq_t[:], bq.rearrange("(t p) -> p t", p=P))
        nc.gpsimd.dma_start(bk_t[:], bk.rearrange("(t p) -> p t", p=P))
        bv_stage = constp.tile([1, D], F32, tag="bstage", bufs=2, name="bv_stage")
        nc.gpsimd.dma_start(bv_stage[:], bv.rearrange("(o d) -> o d", o=1))
        bo_stage = constp.tile([1, D], F32, tag="bstage", bufs=2, name="bo_stage")
        nc.gpsimd.dma_start(bo_stage[:], bo.rearrange("(o d) -> o d", o=1))

        # warm the ACT table set (exp+ln) while DMAs stream
        actw = constp.tile([1, 8], F32, tag="actw")
        nc.vector.memset(actw[:], 1.0)
        nc.scalar.activation(actw[0:1, 0:4], actw[0:1, 4:8], Ln)
        nc.scalar.activation(actw[0:1, 0:4], actw[0:1, 4:8], Exp)

        # ---- small on-chip constants ---------------------------------------
        ones_stage = constp.tile([1, P], F32, tag="onesstage")
        nc.vector.memset(ones_stage[:], 1.0)
        ones_row = constp.tile([1, P], F32R, tag="onesrow")
        nc.vector.tensor_copy(ones_row[:], ones_stage[:])

        bv_row = constp.tile([1, D], F32R, tag="bvrow")
        nc.vector.tensor_copy(bv_row[:], bv_stage[:])
        bo_row = constp.tile([1, D], F32R, tag="borow")
        nc.vector.tensor_copy(bo_row[:], bo_stage[:])

        ones_col = constp.tile([P, H], F32, tag="onescol")
        nc.vector.memset(ones_col[:], 1.0)

        # all-ones F32R rows (only partitions 0/32 are read) for denom bcast
        ones33_stage = constp.tile([33, 64], F32, tag="o33s")
        nc.vector.memset(ones33_stage[:], 1.0)
        ones33 = constp.tile([33, 64], F32R, tag="o33")
        nc.vector.tensor_copy(ones33[:], ones33_stage[:])

        # denominator staging: engine APs need 32-aligned partition bases,
        # so head (2hp+sub)'s rowsums live at [32*sub, hp*QH:(hp+1)*QH].
        # Rows 1..31 are never read; memset keeps Ln's input defined.
        DEN_W = MT * QH
        denoms = [
            constp.tile([33, DEN_W], F32, tag=f"den{pp}", name=f"den{pp}")
            for pp in range(2)
        ]
        for pp in range(2):
            nc.vector.memset(denoms[pp][:], 1.0)
        lnr_t = constp.tile([33, DEN_W], F32, tag="lnrt", name="lnrt")
        frec_t = constp.tile([33, DEN_W], F32R, tag="frect", name="frect")

        # ---- broadcast bias rows to all partitions via K=1 matmul ----------
        bv_b = constp.tile([P, D], F32, tag="bvb")
        bo_b = constp.tile([P, D], F32, tag="bob")

        def bias_bcast(row, bcast):
            for c0, c1 in ((0, 512), (512, 768)):
                bps = psflow.tile([P, 512], F32, tag="ps", name="bps")
                nc.tensor.matmul(
                    bps[:, : c1 - c0],
                    ones_row[:],
                    row[:, c0:c1],
                    start=True,
                    stop=True,
                )
                nc.vector.tensor_copy(bcast[:, c0:c1], bps[:, : c1 - c0])

        # ---- projections ----------------------------------------------------
        qts = [qtp.tile([P, S], MMDT, tag=f"qt{i}", name=f"qt{i}") for i in range(MT)]
        kts = [ktp.tile([P, S], MMDT, tag=f"kt{i}", name=f"kt{i}") for i in range(MT)]
        vps = [
            vpp.tile([P, H * 65], MMDT, tag=f"vp{st}", name=f"vp{st}")
            for st in range(NKT)
        ]

        def proj_qk_gen(w3, b_t, dst, sc, hdb):
            s0 = sc * 512
            ps0 = psflow.tile([P, 512], F32, tag="ps", name="pj0")
            for mt in range(MT):
                nc.tensor.matmul(
                    ps0[:], w3[:, mt, hdb * P : (hdb + 1) * P],
                    xt3[:, mt, s0 : s0 + 512],
                    start=(mt == 0), stop=(mt == MT - 1),
                )
                if mt in (1, 3):
                    yield
            bsl = b_t[:, hdb : hdb + 1]
            bb = bass.AP(bsl.tensor, bsl.offset, [bsl.ap[0], [0, 512]])
            nc.vector.tensor_tensor(
                dst[hdb][:, s0 : s0 + 512], ps0[:], bb, op=add
            )

        def proj_qk_piece(w3, b_t, dst, sc, hdb):
            for _ in proj_qk_gen(w3, b_t, dst, sc, hdb):
                pass

        def proj_v_gen(st):
            vv = vps[st].rearrange("p (h c) -> p h c", c=65)
            nc.vector.tensor_copy(
                vv[:, :, 64:65],
                ones_col.rearrange("p (h c) -> p h c", c=1),
            )
            ps0 = psflow.tile([P, 512], F32, tag="ps", name="pv0")
            ps1 = psflow.tile([P, 512], F32, tag="ps", name="pv1")
            for mt in range(MT):
                lx = xt3[:, mt, st * P : (st + 1) * P]
                nc.tensor.matmul(
                    ps0[:], lx, wv3[:, mt, 0:512],
                    start=(mt == 0), stop=(mt == MT - 1),
                )
                nc.tensor.matmul(
                    ps1[:, 0:256], lx, wv3[:, mt, 512:768],
                    start=(mt == 0), stop=(mt == MT - 1),
                )
                if mt in (1, 3):
                    yield
            bsrc = bv_b.rearrange("p (h c) -> p h c", c=DH)
            nc.vector.tensor_tensor(
                vv[:, 0:8, 0:DH],
                ps0.rearrange("p (h c) -> p h c", c=DH),
                bsrc[:, 0:8, :],
                op=add,
            )
            nc.vector.tensor_tensor(
                vv[:, 8:12, 0:DH],
                ps1[:, 0:256].rearrange("p (h c) -> p h c", c=DH),
                bsrc[:, 8:12, :],
                op=add,
            )

        class Feeder:
            """Doles out deferred emission work in small steps so the PE
            stream interleaves finely with attention matmuls."""

            def __init__(self):
                from collections import deque
                self.q = deque()

            def add(self, gen):
                self.q.append(gen)
                return gen

            def step(self):
                while self.q:
                    try:
                        next(self.q[0])
                        return True
                    except StopIteration:
                        self.q.popleft()
                return False

            def drain_until(self, gen):
                """Emit queued work until `gen` has fully emitted."""
                while any(g is gen for g in self.q):
                    try:
                        next(self.q[0])
                    except StopIteration:
                        self.q.popleft()

            def drain(self):
                while self.step():
                    pass

        feeder = Feeder()

        def attn_core_gen(pp, hp, wstack):
            q0 = pp * QH
            nkt1 = 4 * pp + 4
            pvs = [
                psacc.tile([65, QH], F32, tag="pv", name=f"pv{sub}")
                for sub in range(2)
            ]
            for kt in range(nkt1):
                # left edge of the causal window at 128-col granularity
                c0 = max(0, kt - 4 * pp) * P
                w = QH - c0
                scp = scpp.tile([P, 2 * QH], F32, tag="scp", name="scp")
                # the pair's two matmuls sit on disjoint PE row groups and
                # disjoint PSUM banks of one 2-bank tile
                for sub in range(2):
                    r0 = sub * 64
                    nc.tensor.matmul(
                        scp[:, sub * QH + c0 : (sub + 1) * QH],
                        kts[hp][r0 : r0 + 64, kt * P : (kt + 1) * P],
                        qts[hp][r0 : r0 + 64, q0 + c0 : q0 + QH],
                        start=True,
                        stop=True,
                        tile_position=(r0, 0),
                    )
                est = expp.tile([P, 2 * QH], MMDT, tag="est", name="est")
                if c0 == 0:
                    nc.scalar.activation(
                        est[:], scp[:], Exp, scale=SCALE
                    )
                else:
                    sin = bass.AP(
                        scp.tensor, scp.offset + c0,
                        [scp.ap[0], [QH, 2], [1, w]],
                    )
                    sout = bass.AP(
                        est.tensor, est.offset + c0,
                        [est.ap[0], [QH, 2], [1, w]],
                    )
                    nc.scalar.activation(sout, sin, Exp, scale=SCALE)
                dk = kt - 4 * pp
                if dk >= 0:
                    ms = dk * P
                    for sub in range(2):
                        b0 = sub * QH
                        nc.gpsimd.affine_select(
                            est[:, b0 + ms : b0 + ms + P],
                            est[:, b0 + ms : b0 + ms + P],
                            pattern=[[1, P]],
                            compare_op=is_ge, fill=0.0,
                            base=0,
                            channel_multiplier=-1,
                        )
                last = kt == nkt1 - 1
                for sub in range(2):
                    h = 2 * hp + sub
                    nc.tensor.matmul(
                        pvs[sub][:, c0:QH],
                        vps[kt][:, h * 65 : (h + 1) * 65],
                        est[:, sub * QH + c0 : (sub + 1) * QH],
                        start=(kt == 0),
                        stop=last,
                        skip_group_check=True,
                    )
                    if last:
                        # stash immediately: sub0's copies overlap sub1's
                        # PV on the PE, so the psacc slots free bubble-free
                        r0 = sub * 64
                        nc.vector.tensor_copy(
                            wstack[hp][r0 : r0 + 64, :], pvs[sub][0:64, :]
                        )
                        nc.vector.tensor_copy(
                            denoms[pp][32 * sub : 32 * sub + 1,
                                       hp * QH : (hp + 1) * QH],
                            pvs[sub][64:65, :],
                        )
                yield

        def run_attn(pp, hp, wstack, steps=1):
            for _ in attn_core_gen(pp, hp, wstack):
                for _ in range(steps):
                    feeder.step()

        def norm_apply(pp, hp, wstack):
            for sub in range(2):
                r0 = sub * 64
                rb = psflow.tile([P, 512], F32, tag="ps", name="rb")
                nc.tensor.matmul(
                    rb[0:64, :],
                    ones33[32 * sub : 32 * sub + 1, :],
                    frec_t[32 * sub : 32 * sub + 1,
                           hp * QH : (hp + 1) * QH],
                    start=True, stop=True,
                )
                nc.vector.tensor_tensor(
                    wstack[hp][r0 : r0 + 64, :],
                    wstack[hp][r0 : r0 + 64, :],
                    rb[0:64, :], op=mult,
                )

        def norm_gen(pp, h0, h1, wstack):
            # reciprocal via exp(-ln r) — same ACT table set as softmax exp
            c0, c1 = h0 * QH, h1 * QH
            nc.scalar.activation(
                lnr_t[:, c0:c1], denoms[pp][:, c0:c1], Ln
            )
            nc.scalar.activation(
                frec_t[:, c0:c1], lnr_t[:, c0:c1], Exp, scale=-1.0
            )
            yield
            for hp in range(h0, h1):
                norm_apply(pp, hp, wstack)
                yield

        def outproj_gen(pp, wstack, sub):
            q0 = pp * QH
            opsa = psflow.tile([P, 512], F32, tag="ps", name="opa_t")
            opsb = psflow.tile([P, 512], F32, tag="ps", name="opb_t")
            for hdt in range(MT):
                lw = wstack[hdt][:, sub * P : (sub + 1) * P]
                nc.tensor.matmul(
                    opsa[:], lw, wo3[:, hdt, 0:512],
                    start=(hdt == 0), stop=(hdt == MT - 1),
                )
                nc.tensor.matmul(
                    opsb[:, 0:256], lw, wo3[:, hdt, 512:768],
                    start=(hdt == 0), stop=(hdt == MT - 1),
                )
                if hdt in (1, 3):
                    yield
            osb = outp.tile([P, D], MMDT, tag="osb")
            nc.vector.tensor_tensor(
                osb[:, 0:512], opsa[:], bo_b[:, 0:512], op=add
            )
            nc.vector.tensor_tensor(
                osb[:, 512:768], opsb[:, 0:256], bo_b[:, 512:768], op=add
            )
            eng = nc.sync if (pp == 0 or sub % 2 == 0) else nc.gpsimd
            eng.dma_start(y[q0 + sub * P : q0 + (sub + 1) * P, :], osb[:])

        # ---- emission: attention as backbone, all else fed between k-tiles
        wstack0 = [
            wsp.tile([P, QH], MMDT, tag="ws", name=f"ws0_{i}")
            for i in range(MT)
        ]
        wstack1 = [
            wsp.tile([P, QH], MMDT, tag="ws", name=f"ws1_{i}")
            for i in range(MT)
        ]

        # prelude: exactly what attn(0,0) needs
        proj_qk_piece(wq3, bq_t, qts, 0, 0)
        bias_bcast(bv_row, bv_b)
        bias_bcast(bo_row, bo_b)
        proj_qk_piece(wk3, bk_t, kts, 0, 0)
        for st in range(4):
            for _ in proj_v_gen(st):
                pass

        k0_gens = {}
        k1_gens = {}
        for hp in range(MT):
            if hp < MT - 1:
                feeder.add(proj_qk_gen(wq3, bq_t, qts, 0, hp + 1))
                k0_gens[hp + 1] = feeder.add(
                    proj_qk_gen(wk3, bk_t, kts, 0, hp + 1)
                )
            if hp < 4:
                feeder.add(proj_v_gen(4 + hp))
            if hp in k0_gens:
                feeder.drain_until(k0_gens[hp])
            run_attn(0, hp, wstack0, steps=2)

        # stage half-1 projections for the first two head-pairs, then let
        # the pp1 attention stream pace the rest
        feeder.add(proj_qk_gen(wq3, bq_t, qts, 1, 0))
        k1_gens[0] = feeder.add(proj_qk_gen(wk3, bk_t, kts, 1, 0))
        feeder.add(proj_qk_gen(wq3, bq_t, qts, 1, 1))
        k1_gens[1] = feeder.add(proj_qk_gen(wk3, bk_t, kts, 1, 1))
        feeder.drain_until(k1_gens[0])

        for hp in range(MT):
            if hp < 4:
                feeder.add(proj_qk_gen(wq3, bq_t, qts, 1, hp + 2))
                k1_gens[hp + 2] = feeder.add(
                    proj_qk_gen(wk3, bk_t, kts, 1, hp + 2)
                )
            if hp == 0:
                feeder.add(norm_gen(0, 0, MT, wstack0))
            elif hp <= 4:
                feeder.add(outproj_gen(0, wstack0, hp - 1))
            else:
                feeder.add(norm_gen(1, 0, 5, wstack1))
            feeder.drain_until(k1_gens[hp])
            run_attn(1, hp, wstack1, steps=1)

        feeder.drain()
        for _ in norm_gen(1, 5, MT, wstack1):
            pass
        for sub in range(4):
            for _ in outproj_gen(1, wstack1, sub):
                pass
    return nc


_NC_CACHE = None
LAST_EXEC_NS = None


def _get_nc():
    global _NC_CACHE
    if _NC_CACHE is None:
        _NC_CACHE = build_nc()
    return _NC_CACHE


def kernel(
    normalized_resid_pre, W_Q, W_K, W_V, W_O, b_Q, b_K, b_V, b_O
) -> np.ndarray:
    global LAST_EXEC_NS
    bf = ml_dtypes.bfloat16
    x = np.asarray(normalized_resid_pre, np.float32)
    xT = np.ascontiguousarray(x.transpose(0, 2, 1)).astype(bf)  # [b, D, S]
    wq = np.asarray(W_Q, np.float32).transpose(1, 0, 2).reshape(D, D).astype(bf)
    wk = np.asarray(W_K, np.float32).transpose(1, 0, 2).reshape(D, D).astype(bf)
    wv = np.asarray(W_V, np.float32).transpose(1, 0, 2).reshape(D, D).astype(bf)
    wo = np.asarray(W_O, np.float32).reshape(D, D).astype(bf)
    bq = np.asarray(b_Q, np.float32).reshape(D).copy()
    bk = np.asarray(b_K, np.float32).reshape(D).copy()
    bv = np.asarray(b_V, np.float32).reshape(D).copy()
    bo = np.asarray(b_O, np.float32).reshape(D).copy()

    nc = _get_nc()
    in_maps = [
        {
            "xT": xT[i],
            "wq": wq, "wk": wk, "wv": wv, "wo": wo,
            "bq": bq, "bk": bk, "bv": bv, "bo": bo,
        }
        for i in range(N_CORES)
    ]
    trace = os.environ.get("KERNEL_TRACE", "0") == "1"
    res = run_bass_kernel_spmd(
        nc, in_maps, list(range(N_CORES)), trace=trace
    )
    LAST_EXEC_NS = res.exec_time_ns
    out = np.stack(
        [res.results[i]["y"].astype(np.float32) for i in range(N_CORES)], axis=0
    )
    return out


# revision 11
# speedup vs baseline: 1.2359x; 1.0915x over previous
"""Causal multi-head attention block (b=8, s=1024, d_model=768, 12 heads x 64)
on 8 TRN2 NeuronCores — batch-parallel: core i computes batch element i.

Self-contained: includes the NTFF-profile-hook shim and the BIR wait-split
workaround for this walrus build (max 1 semaphore wait per instruction).

Per-core plan (bf16 matmuls, fp32 PSUM accumulation):
  A. x arrives bf16 (host-transposed); x / W_Q land in two big DMAs each on
     the sync (HWDGE) queue, W_V/W_K/W_O on the scalar queue, biases on
     gpsimd — few large transfers, issue-order = first-use order.
  B. QT/KT [hd-blk][128,1024] = W.T @ xT (head-pair packed); V in natural
     [s,hd] layout padded to 65 cols/head with a ones column (rowsum trick)
  C. per q-half(512) / head-pair: scoresT[k,q] = KT.T @ QT on PE (left-
     trimmed to the causal window), exp on ACT (1/8 scale folded in),
     causal mask via gpsimd affine_select on [128,128] diagonal windows,
     PV accumulates [65,512]x2 (row 64 = softmax denominator).
     Denominators land at partitions {0,32} of a [33,3072] tile; one
     batched Ln+Exp pair per q-half computes reciprocals; K=1 ones-row
     matmuls broadcast them and DVE multiplies normalize the stacks.
  D. out-proj + b_O in bf16; y DMA'd bf16 (host upcasts to f32).
  Attention is the emission backbone from the start; every other matmul
  (projections, out-proj, normalization broadcast) is doled out by a
  feeder between attention k-tiles so the in-order PE queue always holds
  runnable work — keeping the PE HAM clock-gate warm through the
  ACT-paced stretches.
"""

import os
import sys
import types

import numpy as np

# ---------------------------------------------------------------------------
# environment shims


def _install_ntff_hook():
    try:
        import antenv
        from trn_agent_boot.trn_boot import _ntff_profile_via_ctypes
    except Exception:
        return
    if "antenv.axon_hooks" in sys.modules:
        return
    hook = _ntff_profile_via_ctypes("/opt/axon/libaxon_pjrt.so")
    m = types.ModuleType("antenv.axon_hooks")
    m.set_axon_ntff_profile_hook = lambda h: None
    m.get_axon_ntff_profile_hook = lambda: hook
    sys.modules["antenv.axon_hooks"] = m
    antenv.axon_hooks = m


def _install_waitsplit(max_waits=1):
    """walrus on this build rejects >1 sem wait per instruction; split extras
    onto preceding NoOps (same engine, program order preserved)."""
    import json

    import concourse.bass as bass

    if getattr(bass.Bass, "_waitsplit_installed", False):
        return
    counter = [0]

    def _split(inst):
        si = inst.get("sync_info")
        if not si:
            return [inst]
        waits = si.get("on_wait") or []
        if len(waits) <= max_waits:
            return [inst]
        out = []
        head, rest = waits[:-max_waits], waits[-max_waits:]
        for i in range(0, len(head), max_waits):
            counter[0] += 1
            out.append(
                {
                    "debug": inst.get("debug", 0),
                    "engine": inst["engine"],
                    "ins": [],
                    "name": f"I-waitsplit-{counter[0]}",
                    "opcode": "NoOp",
                    "outs": [],
                    "text_hint": "waitsplit",
                    "sync_info": {
                        "on_update": [],
                        "on_wait": head[i : i + max_waits],
                    },
                }
            )
        si["on_wait"] = rest
        out.append(inst)
        return out

    orig = bass.Bass.to_json_bytes

    def to_json_bytes(self):
        d = json.loads(orig(self))
        changed = False
        for f in d.get("functions", []):
            for bb in f.get("blocks", []):
                new = []
                for inst in bb.get("instructions", []):
                    parts = _split(inst)
                    changed = changed or len(parts) > 1
                    new.extend(parts)
                bb["instructions"] = new
        return json.dumps(d).encode() if changed else orig(self)

    bass.Bass.to_json_bytes = to_json_bytes
    bass.Bass._waitsplit_installed = True


_install_ntff_hook()
_install_waitsplit()

import ml_dtypes  # noqa: E402
import concourse.bass as bass  # noqa: E402
import concourse.mybir as mybir  # noqa: E402
import concourse.tile as tile  # noqa: E402
from concourse.bass_utils import run_bass_kernel_spmd  # noqa: E402

# ---------------------------------------------------------------------------
# problem constants (hardcoded per harness contract)

B, S, D, H, DH = 8, 1024, 768, 12, 64
P = 128
MT = D // P            # 6 tiles over d_model / hd
QC = 256               # q-chunk width
QH = 512               # q-half (pair of chunks)
NKT = S // P           # 8 k-tiles over seq
SCALE = float(1.0 / np.sqrt(DH))
N_CORES = 8

F32 = mybir.dt.float32
F32R = mybir.dt.float32r
BF16 = mybir.dt.bfloat16
F8 = mybir.dt.float8e4
MMDT = BF16
WS = 16.0                     # host scale on W_Q/W_K/b_Q/b_K before fp8
SCALE8 = SCALE / (WS * WS)    # folded into the softmax exp


def build_nc() -> bass.Bass:
    nc = bass.Bass()
    xT = nc.declare_dram_parameter("xT", [D, S], MMDT, isOutput=False)
    x8d = nc.declare_dram_parameter("x8", [P, MT * S], F8, isOutput=False)
    wq8d = nc.declare_dram_parameter("wq8", [P, MT * D], F8, isOutput=False)
    wk8d = nc.declare_dram_parameter("wk8", [P, MT * D], F8, isOutput=False)
    wv = nc.declare_dram_parameter("wv", [D, D], MMDT, isOutput=False)
    wo = nc.declare_dram_parameter("wo", [D, D], MMDT, isOutput=False)
    bq = nc.declare_dram_parameter("bq", [D], F32, isOutput=False)
    bk = nc.declare_dram_parameter("bk", [D], F32, isOutput=False)
    bv = nc.declare_dram_parameter("bv", [D], F32, isOutput=False)
    bo = nc.declare_dram_parameter("bo", [D], F32, isOutput=False)
    y = nc.declare_dram_parameter("y", [S, D], MMDT, isOutput=True)

    Exp = mybir.ActivationFunctionType.Exp
    Ln = mybir.ActivationFunctionType.Ln
    mult = mybir.AluOpType.mult
    add = mybir.AluOpType.add
    is_ge = mybir.AluOpType.is_ge

    from contextlib import ExitStack

    with ExitStack() as _ctx:
        tc = _ctx.enter_context(tile.TileContext(nc))
        constp = _ctx.enter_context(tc.tile_pool(name="const", bufs=1))
        xtp = _ctx.enter_context(tc.tile_pool(name="xT", bufs=1))
        qtp = _ctx.enter_context(tc.tile_pool(name="qt", bufs=1))
        ktp = _ctx.enter_context(tc.tile_pool(name="kt", bufs=1))
        vpp = _ctx.enter_context(tc.tile_pool(name="vp", bufs=1))
        wtsp = _ctx.enter_context(tc.tile_pool(name="wts", bufs=1))
        expp = _ctx.enter_context(tc.tile_pool(name="expst", bufs=10))
        wsp = _ctx.enter_context(tc.tile_pool(name="wstack", bufs=12))
        outp = _ctx.enter_context(tc.tile_pool(name="outsb", bufs=2))
        psflow = _ctx.enter_context(
            tc.tile_pool(name="ps_flow", bufs=2, space="PSUM")
        )
        psacc = _ctx.enter_context(
            tc.tile_pool(name="ps_acc", bufs=2, space="PSUM")
        )
        scpp = _ctx.enter_context(
            tc.tile_pool(name="ps_scp", bufs=2, space="PSUM")
        )

        # ---- input DMAs: one priority stream on the sync queue --------------
        # order = first compute need: fp8 Q path, V path, fp8 K, rest of x, W_O
        x8t = xtp.tile([P, MT * S], F8, tag="x8t", name="x8t")
        x8v = x8t.rearrange("p (t s) -> p t s", s=S)
        wq8t = wtsp.tile([P, MT * D], F8, tag="wq8", name="wq8")
        wq8v = wq8t.rearrange("p (t c) -> p t c", c=D)
        wk8t = wtsp.tile([P, MT * D], F8, tag="wk8", name="wk8")
        wk8v = wk8t.rearrange("p (t c) -> p t c", c=D)
        xt_all = xtp.tile([P, MT * S], MMDT, tag="xta", name="xta")
        xt3 = xt_all.rearrange("p (t s) -> p t s", s=S)
        xTv = xT.rearrange("(t p) s -> p t s", p=P)

        def wtile(name):
            t = wtsp.tile([P, MT * D], MMDT, tag=name, name=name)
            return t, t.rearrange("p (t c) -> p t c", c=D)

        wv_all, wv3 = wtile("wvt")
        wo_all, wo3 = wtile("wot")

        nc.sync.dma_start(x8t[:], x8d[:, :])
        nc.sync.dma_start(wq8t[:], wq8d[:, :])
        wvv = wv.rearrange("(t p) c -> p t c", p=P)
        nc.sync.dma_start(wv3[:, 0:3, :], wvv[:, 0:3, :])
        nc.sync.dma_start(wv3[:, 3:6, :], wvv[:, 3:6, :])
        nc.sync.dma_start(xt3[:, :, 0:512], xTv[:, :, 0:512])
        nc.sync.dma_start(wk8t[:], wk8d[:, :])
        nc.sync.dma_start(xt3[:, :, 512:1024], xTv[:, :, 512:1024])
        wov = wo.rearrange("(t p) c -> p t c", p=P)
        nc.scalar.dma_start(wo3[:, 0:3, :], wov[:, 0:3, :])
        nc.scalar.dma_start(wo3[:, 3:6, :], wov[:, 3:6, :])

        # gpsimd queue: bias constants (one strided DMA per bias vector)
        bq_t = constp.tile([P, MT], F32, tag="bq")  # col hdb = bias block
        bk_t = constp.tile([P, MT], F32, tag="bk")
        nc.gpsimd.dma_start(bq_t[:], bq.rearrange("(t p) -> p t", p=P))
        nc.gpsimd.dma_start(bk_t[:], bk.rearrange("(t p) -> p t", p=P))
        bv_stage = constp.tile([1, D], F32, tag="bstage", bufs=2, name="bv_stage")
        nc.gpsimd.dma_start(bv_stage[:], bv.rearrange("(o d) -> o d", o=1))
        bo_stage = constp.tile([1, D], F32, tag="bstage", bufs=2, name="bo_stage")
        nc.gpsimd.dma_start(bo_stage[:], bo.rearrange("(o d) -> o d", o=1))

        # warm the ACT table set (exp+ln) while DMAs stream
        actw = constp.tile([1, 8], F32, tag="actw")
        nc.vector.memset(actw[:], 1.0)
        nc.scalar.activation(actw[0:1, 0:4], actw[0:1, 4:8], Ln)
        nc.scalar.activation(actw[0:1, 0:4], actw[0:1, 4:8], Exp)

        # ---- small on-chip constants ---------------------------------------
        ones_stage = constp.tile([1, P], F32, tag="onesstage")
        nc.vector.memset(ones_stage[:], 1.0)
        ones_row = constp.tile([1, P], F32R, tag="onesrow")
        nc.vector.tensor_copy(ones_row[:], ones_stage[:])

        bv_row = constp.tile([1, D], F32R, tag="bvrow")
        nc.vector.tensor_copy(bv_row[:], bv_stage[:])
        bo_row = constp.tile([1, D], F32R, tag="borow")
        nc.vector.tensor_copy(bo_row[:], bo_stage[:])

        ones_col = constp.tile([P, H], F32, tag="onescol")
        nc.vector.memset(ones_col[:], 1.0)

        # all-ones F32R rows (only partitions 0/32 are read) for denom bcast
        ones33_stage = constp.tile([33, 64], F32, tag="o33s")
        nc.vector.memset(ones33_stage[:], 1.0)
        ones33 = constp.tile([33, 64], F32R, tag="o33")
        nc.vector.tensor_copy(ones33[:], ones33_stage[:])

        # denominator staging: engine APs need 32-aligned partition bases,
        # so head (2hp+sub)'s rowsums live at [32*sub, hp*QH:(hp+1)*QH].
        # Rows 1..31 are never read; memset keeps Ln's input defined.
        DEN_W = MT * QH
        denoms = [
            constp.tile([33, DEN_W], F32, tag=f"den{pp}", name=f"den{pp}")
            for pp in range(2)
        ]
        for pp in range(2):
            nc.vector.memset(denoms[pp][:], 1.0)
        lnr_t = constp.tile([33, DEN_W], F32, tag="lnrt", name="lnrt")
        frec_t = constp.tile([33, DEN_W], F32R, tag="frect", name="frect")

        # ---- broadcast bias rows to all partitions via K=1 matmul ----------
        bv_b = constp.tile([P, D], F32, tag="bvb")
        bo_b = constp.tile([P, D], F32, tag="bob")

        def bias_bcast(row, bcast):
            for c0, c1 in ((0, 512), (512, 768)):
                bps = psflow.tile([P, 512], F32, tag="ps", name="bps")
                nc.tensor.matmul(
                    bps[:, : c1 - c0],
                    ones_row[:],
                    row[:, c0:c1],
                    start=True,
                    stop=True,
                )
                nc.vector.tensor_copy(bcast[:, c0:c1], bps[:, : c1 - c0])

        # ---- projections ----------------------------------------------------
        qts = [qtp.tile([P, S], MMDT, tag=f"qt{i}", name=f"qt{i}") for i in range(MT)]
        kts = [ktp.tile([P, S], MMDT, tag=f"kt{i}", name=f"kt{i}") for i in range(MT)]
        vps = [
            vpp.tile([P, H * 65], MMDT, tag=f"vp{st}", name=f"vp{st}")
            for st in range(NKT)
        ]

        def proj_qk_gen(w8v, b_t, dst, sc, hdb):
            s0 = sc * 512
            ps0 = psflow.tile([P, 512], F32, tag="ps", name="pj0")
            for t in range(3):
                nc.tensor.matmul(
                    ps0[:], w8v[:, 2 * t : 2 * t + 2, hdb * P : (hdb + 1) * P],
                    x8v[:, 2 * t : 2 * t + 2, s0 : s0 + 512],
                    start=(t == 0), stop=(t == 2),
                    perf_mode=mybir.MatmulPerfMode.DoubleRow,
                )
                if t in (0, 1):
                    yield
            bsl = b_t[:, hdb : hdb + 1]
            bb = bass.AP(bsl.tensor, bsl.offset, [bsl.ap[0], [0, 512]])
            nc.vector.tensor_tensor(
                dst[hdb][:, s0 : s0 + 512], ps0[:], bb, op=add
            )

        def proj_qk_piece(w3, b_t, dst, sc, hdb):
            for _ in proj_qk_gen(w3, b_t, dst, sc, hdb):
                pass

        def proj_v_gen(st):
            vv = vps[st].rearrange("p (h c) -> p h c", c=65)
            nc.vector.tensor_copy(
                vv[:, :, 64:65],
                ones_col.rearrange("p (h c) -> p h c", c=1),
            )
            ps0 = psflow.tile([P, 512], F32, tag="ps", name="pv0")
            ps1 = psflow.tile([P, 512], F32, tag="ps", name="pv1")
            for mt in range(MT):
                lx = xt3[:, mt, st * P : (st + 1) * P]
                nc.tensor.matmul(
                    ps0[:], lx, wv3[:, mt, 0:512],
                    start=(mt == 0), stop=(mt == MT - 1),
                )
                nc.tensor.matmul(
                    ps1[:, 0:256], lx, wv3[:, mt, 512:768],
                    start=(mt == 0), stop=(mt == MT - 1),
                )
                if mt in (1, 3):
                    yield
            bsrc = bv_b.rearrange("p (h c) -> p h c", c=DH)
            nc.vector.tensor_tensor(
                vv[:, 0:8, 0:DH],
                ps0.rearrange("p (h c) -> p h c", c=DH),
                bsrc[:, 0:8, :],
                op=add,
            )
            nc.vector.tensor_tensor(
                vv[:, 8:12, 0:DH],
                ps1[:, 0:256].rearrange("p (h c) -> p h c", c=DH),
                bsrc[:, 8:12, :],
                op=add,
            )

        class Feeder:
            """Doles out deferred emission work in small steps so the PE
            stream interleaves finely with attention matmuls."""

            def __init__(self):
                from collections import deque
                self.q = deque()

            def add(self, gen):
                self.q.append(gen)
                return gen

            def step(self):
                while self.q:
                    try:
                        next(self.q[0])
                        return True
                    except StopIteration:
                        self.q.popleft()
                return False

            def drain_until(self, gen):
                """Emit queued work until `gen` has fully emitted."""
                while any(g is gen for g in self.q):
                    try:
                        next(self.q[0])
                    except StopIteration:
                        self.q.popleft()

            def drain(self):
                while self.step():
                    pass

        feeder = Feeder()

        def attn_core_gen(pp, hp, wstack):
            q0 = pp * QH
            nkt1 = 4 * pp + 4
            pvs = [
                psacc.tile([65, QH], F32, tag="pv", name=f"pv{sub}")
                for sub in range(2)
            ]
            for kt in range(nkt1):
                # left edge of the causal window at 128-col granularity
                c0 = max(0, kt - 4 * pp) * P
                w = QH - c0
                scp = scpp.tile([P, 2 * QH], F32, tag="scp", name="scp")
                # the pair's two matmuls sit on disjoint PE row groups and
                # disjoint PSUM banks of one 2-bank tile
                for sub in range(2):
                    r0 = sub * 64
                    nc.tensor.matmul(
                        scp[:, sub * QH + c0 : (sub + 1) * QH],
                        kts[hp][r0 : r0 + 64, kt * P : (kt + 1) * P],
                        qts[hp][r0 : r0 + 64, q0 + c0 : q0 + QH],
                        start=True,
                        stop=True,
                        tile_position=(r0, 0),
                    )
                est = expp.tile([P, 2 * QH], MMDT, tag="est", name="est")
                if c0 == 0:
                    nc.scalar.activation(
                        est[:], scp[:], Exp, scale=SCALE8
                    )
                else:
                    sin = bass.AP(
                        scp.tensor, scp.offset + c0,
                        [scp.ap[0], [QH, 2], [1, w]],
                    )
                    sout = bass.AP(
                        est.tensor, est.offset + c0,
                        [est.ap[0], [QH, 2], [1, w]],
                    )
                    nc.scalar.activation(sout, sin, Exp, scale=SCALE8)
                dk = kt - 4 * pp
                if dk >= 0:
                    ms = dk * P
                    for sub in range(2):
                        b0 = sub * QH
                        nc.gpsimd.affine_select(
                            est[:, b0 + ms : b0 + ms + P],
                            est[:, b0 + ms : b0 + ms + P],
                            pattern=[[1, P]],
                            compare_op=is_ge, fill=0.0,
                            base=0,
                            channel_multiplier=-1,
                        )
                last = kt == nkt1 - 1
                for sub in range(2):
                    h = 2 * hp + sub
                    nc.tensor.matmul(
                        pvs[sub][:, c0:QH],
                        vps[kt][:, h * 65 : (h + 1) * 65],
                        est[:, sub * QH + c0 : (sub + 1) * QH],
                        start=(kt == 0),
                        stop=last,
                        skip_group_check=True,
                    )
                    if last:
                        # stash immediately: sub0's copies overlap sub1's
                        # PV on the PE, so the psacc slots free bubble-free
                        r0 = sub * 64
                        nc.vector.tensor_copy(
                            wstack[hp][r0 : r0 + 64, :], pvs[sub][0:64, :]
                        )
                        nc.vector.tensor_copy(
                            denoms[pp][32 * sub : 32 * sub + 1,
                                       hp * QH : (hp + 1) * QH],
                            pvs[sub][64:65, :],
                        )
                yield

        def run_attn(pp, hp, wstack, steps=1):
            for _ in attn_core_gen(pp, hp, wstack):
                for _ in range(steps):
                    feeder.step()

        def norm_apply(pp, hp, wstack):
            for sub in range(2):
                r0 = sub * 64
                rb = psflow.tile([P, 512], F32, tag="ps", name="rb")
                nc.tensor.matmul(
                    rb[0:64, :],
                    ones33[32 * sub : 32 * sub + 1, :],
                    frec_t[32 * sub : 32 * sub + 1,
                           hp * QH : (hp + 1) * QH],
                    start=True, stop=True,
                )
                nc.vector.tensor_tensor(
                    wstack[hp][r0 : r0 + 64, :],
                    wstack[hp][r0 : r0 + 64, :],
                    rb[0:64, :], op=mult,
                )

        def norm_gen(pp, h0, h1, wstack):
            # reciprocal via exp(-ln r) — same ACT table set as softmax exp
            c0, c1 = h0 * QH, h1 * QH
            nc.scalar.activation(
                lnr_t[:, c0:c1], denoms[pp][:, c0:c1], Ln
            )
            nc.scalar.activation(
                frec_t[:, c0:c1], lnr_t[:, c0:c1], Exp, scale=-1.0
            )
            yield
            for hp in range(h0, h1):
                norm_apply(pp, hp, wstack)
                yield

        def outproj_gen(pp, wstack, sub):
            q0 = pp * QH
            opsa = psflow.tile([P, 512], F32, tag="ps", name="opa_t")
            opsb = psflow.tile([P, 512], F32, tag="ps", name="opb_t")
            for hdt in range(MT):
                lw = wstack[hdt][:, sub * P : (sub + 1) * P]
                nc.tensor.matmul(
                    opsa[:], lw, wo3[:, hdt, 0:512],
                    start=(hdt == 0), stop=(hdt == MT - 1),
                )
                nc.tensor.matmul(
                    opsb[:, 0:256], lw, wo3[:, hdt, 512:768],
                    start=(hdt == 0), stop=(hdt == MT - 1),
                )
                if hdt in (1, 3):
                    yield
            osb = outp.tile([P, D], MMDT, tag="osb")
            nc.vector.tensor_tensor(
                osb[:, 0:512], opsa[:], bo_b[:, 0:512], op=add
            )
            nc.vector.tensor_tensor(
                osb[:, 512:768], opsb[:, 0:256], bo_b[:, 512:768], op=add
            )
            eng = nc.sync if (pp == 0 or sub % 2 == 0) else nc.gpsimd
            eng.dma_start(y[q0 + sub * P : q0 + (sub + 1) * P, :], osb[:])

        # ---- emission: attention as backbone, all else fed between k-tiles
        wstack0 = [
            wsp.tile([P, QH], MMDT, tag="ws", name=f"ws0_{i}")
            for i in range(MT)
        ]
        wstack1 = [
            wsp.tile([P, QH], MMDT, tag="ws", name=f"ws1_{i}")
            for i in range(MT)
        ]

        # prelude: exactly what attn(0,0) needs
        proj_qk_piece(wq8v, bq_t, qts, 0, 0)
        bias_bcast(bv_row, bv_b)
        bias_bcast(bo_row, bo_b)
        proj_qk_piece(wk8v, bk_t, kts, 0, 0)
        for st in range(4):
            for _ in proj_v_gen(st):
                pass

        k0_gens = {}
        k1_gens = {}
        for hp in range(MT):
            if hp < MT - 1:
                feeder.add(proj_qk_gen(wq8v, bq_t, qts, 0, hp + 1))
                k0_gens[hp + 1] = feeder.add(
                    proj_qk_gen(wk8v, bk_t, kts, 0, hp + 1)
                )
            if hp < 4:
                feeder.add(proj_v_gen(4 + hp))
            if hp in k0_gens:
                feeder.drain_until(k0_gens[hp])
            run_attn(0, hp, wstack0, steps=2)

        # stage half-1 projections for the first two head-pairs, then let
        # the pp1 attention stream pace the rest
        feeder.add(proj_qk_gen(wq8v, bq_t, qts, 1, 0))
        k1_gens[0] = feeder.add(proj_qk_gen(wk8v, bk_t, kts, 1, 0))
        feeder.add(proj_qk_gen(wq8v, bq_t, qts, 1, 1))
        k1_gens[1] = feeder.add(proj_qk_gen(wk8v, bk_t, kts, 1, 1))
        feeder.drain_until(k1_gens[0])

        for hp in range(MT):
            if hp < 4:
                feeder.add(proj_qk_gen(wq8v, bq_t, qts, 1, hp + 2))
                k1_gens[hp + 2] = feeder.add(
                    proj_qk_gen(wk8v, bk_t, kts, 1, hp + 2)
                )
            if hp == 0:
                feeder.add(norm_gen(0, 0, MT, wstack0))
            elif hp <= 4:
                feeder.add(outproj_gen(0, wstack0, hp - 1))
            else:
                feeder.add(norm_gen(1, 0, 5, wstack1))
            feeder.drain_until(k1_gens[hp])
            run_attn(1, hp, wstack1, steps=1)

        feeder.drain()
        for _ in norm_gen(1, 5, MT, wstack1):
            pass
        for sub in range(4):
            for _ in outproj_gen(1, wstack1, sub):
                pass
    return nc


_NC_CACHE = None
LAST_EXEC_NS = None


def _get_nc():
    global _NC_CACHE
    if _NC_CACHE is None:
        _NC_CACHE = build_nc()
    return _NC_CACHE


def kernel(
    normalized_resid_pre, W_Q, W_K, W_V, W_O, b_Q, b_K, b_V, b_O
) -> np.ndarray:
    global LAST_EXEC_NS
    bf = ml_dtypes.bfloat16
    f8 = ml_dtypes.float8_e4m3
    x = np.asarray(normalized_resid_pre, np.float32)
    xTf = np.ascontiguousarray(x.transpose(0, 2, 1))  # [b, D, S]
    xT = xTf.astype(bf)
    x8 = np.ascontiguousarray(
        xTf.reshape(B, MT, P, S).transpose(0, 2, 1, 3)
    ).reshape(B, P, MT * S).astype(f8)
    wqf = np.asarray(W_Q, np.float32).transpose(1, 0, 2).reshape(D, D) * WS
    wkf = np.asarray(W_K, np.float32).transpose(1, 0, 2).reshape(D, D) * WS
    wq8 = np.ascontiguousarray(
        wqf.reshape(MT, P, D).transpose(1, 0, 2)
    ).reshape(P, MT * D).astype(f8)
    wk8 = np.ascontiguousarray(
        wkf.reshape(MT, P, D).transpose(1, 0, 2)
    ).reshape(P, MT * D).astype(f8)
    wv = np.asarray(W_V, np.float32).transpose(1, 0, 2).reshape(D, D).astype(bf)
    wo = np.asarray(W_O, np.float32).reshape(D, D).astype(bf)
    bq = (np.asarray(b_Q, np.float32).reshape(D) * WS).copy()
    bk = (np.asarray(b_K, np.float32).reshape(D) * WS).copy()
    bv = np.asarray(b_V, np.float32).reshape(D).copy()
    bo = np.asarray(b_O, np.float32).reshape(D).copy()

    nc = _get_nc()
    in_maps = [
        {
            "xT": xT[i], "x8": x8[i],
            "wq8": wq8, "wk8": wk8, "wv": wv, "wo": wo,
            "bq": bq, "bk": bk, "bv": bv, "bo": bo,
        }
        for i in range(N_CORES)
    ]
    trace = os.environ.get("KERNEL_TRACE", "0") == "1"
    res = run_bass_kernel_spmd(
        nc, in_maps, list(range(N_CORES)), trace=trace
    )
    LAST_EXEC_NS = res.exec_time_ns
    out = np.stack(
        [res.results[i]["y"].astype(np.float32) for i in range(N_CORES)], axis=0
    )
    return out
